# revision 1
# baseline (speedup 1.0000x reference)
"""EdgeCrossingsLoss Trainium2 kernel (8-core SPMD, data-parallel over query faces).

Two device launches (this bedrock runtime ships no Q7 extended-instruction
ucode, so there is no usable on-device gather; the host does the small
index-merge + geometry gather between the launches):

prog1 (per core, 1280 query rows = 10 tiles of 128):
  PE:  -d2[q, c] = 2*bary_q.bary_c - sq_q - sq_c for all 10240 candidates via a
       K=16 bf16 hi/lo-split matmul (bf16 products are exact, accumulated in
       f32 PSUM -> f32-quality d2). rhs sits in four 16-partition bands at
       base partitions 0/32/64/96 (PE row-tiles) so its DMA is wide.
  ACT: copies each PSUM block into a linear [128, 10240] SBUF -d2 row block.
  DVE: per 2560-chunk, max8 (top-8 values) + max_index (in-chunk positions).
       Output [128, 32] values + indices per tile.

host: exact top-16 merge of the 4 chunk-top-8s per row (lexsort by value desc /
      index asc = the jax top_k tie-break). Rows where a chunk's reported 8
      values all rank above our 16th (the chunk could hide a 9th member of the
      true top-16) are recomputed exactly on the host (vectorized, ~10% of
      rows). Gathers the 16 neighbor faces' edge geometry; folds probabilities
      and the self-neighbor mask into per-(row, slot) weights.

prog2 (per core): all 1280x16 3x3 line-line crossing tests in one batch of
      broadcast-AP tensor ops on DVE (Pool rejects broadcast APs, ACT
      replicates the query geometry), hit = num^2 < EPS^2*|cross|^2 (den=0 /
      NaN cases fall out correctly), weight-masked and reduced per row.

Host sums the 8 per-core partials and divides by num_faces.
"""
import os
import numpy as np
import ml_dtypes
from contextlib import ExitStack

import concourse.bass as bass
import concourse.tile as tile
import concourse.bacc as bacc
from concourse import mybir
from concourse.bass_utils import run_bass_kernel_spmd

F32 = mybir.dt.float32
BF16 = mybir.dt.bfloat16
U16 = mybir.dt.uint16

NCORES = 8
KNN = 16
EPS = 1e-5
FP = 10240            # padded candidate count
NR = FP // NCORES     # 1280 rows per core
NT = NR // 128        # 10 tiles of 128 rows
KMM = 16              # matmul contraction rows (bf16 hi/lo split)
NGRP = 4              # rhs partition bands (at partitions 0/32/64/96)
GW = FP // NGRP       # 2560
PSW = GW // 2         # 1280-wide PSUM tiles (3 banks)
MMCH = 512            # matmul N per instruction (one PSUM bank)
MXCH = 2560           # max8/max_index chunk in SBUF
NCH = FP // MXCH      # 4 chunks
NC8 = NCH * 8         # 40 chunk-top-8 candidates per row
GPS = 10              # prog2: slots [0:GPS) on DVE, [GPS:16) on GPSIMD

ALU = mybir.AluOpType


def _build_prog1():
    nc = bacc.Bacc("TRN2", target_bir_lowering=False, debug=False,
                   num_devices=NCORES)
    # band b occupies partitions [32b, 32b+16); lhsT replicated into each band
    lhsT_in = nc.dram_tensor("lhsT", [128, NR], BF16, kind="ExternalInput").ap()
    rhs_in = nc.dram_tensor("rhs", [128, GW], BF16, kind="ExternalInput").ap()
    cv_out = nc.dram_tensor("cv", [NT, 128, NC8], F32, kind="ExternalOutput").ap()
    ci_out = nc.dram_tensor("ci", [NT, 128, NC8], U16, kind="ExternalOutput").ap()

    with tile.TileContext(nc) as tc, ExitStack() as ctx:
        const_pool = ctx.enter_context(tc.tile_pool(name="const", bufs=1))
        psum_pool = ctx.enter_context(tc.tile_pool(name="psum", bufs=2, space="PSUM"))
        negd2_pool = ctx.enter_context(tc.tile_pool(name="negd2", bufs=2))
        out_pool = ctx.enter_context(tc.tile_pool(name="out", bufs=2))

        lhsT_sb = const_pool.tile([128, NR], BF16)
        nc.sync.dma_start(lhsT_sb[:], lhsT_in[:])
        rhs_sb = const_pool.tile([128, GW], BF16)
        for j in range(4):   # column chunks on two queues: matmuls start early
            eng = (nc.scalar, nc.sync)[j % 2]
            eng.dma_start(rhs_sb[:, j * (GW // 4):(j + 1) * (GW // 4)],
                          rhs_in[:, j * (GW // 4):(j + 1) * (GW // 4)])

        for t in range(NT):
            negd2 = negd2_pool.tile([128, FP], F32, tag="negd2")
            cv = out_pool.tile([128, NC8], F32, tag="cv")
            ci = out_pool.tile([128, NC8], U16, tag="ci")
            for g in range(NGRP):
                for h in range(GW // PSW):
                    ps = psum_pool.tile([128, PSW], F32, tag="ps")
                    base = h * PSW
                    for c0 in range(base, base + PSW, MMCH):
                        n = min(MMCH, base + PSW - c0)
                        nc.tensor.matmul(
                            ps[:, c0 - base:c0 - base + n],
                            lhsT=lhsT_sb[32 * g:32 * g + KMM,
                                         t * 128:(t + 1) * 128],
                            rhs=rhs_sb[32 * g:32 * g + KMM, c0:c0 + n],
                            start=True, stop=True,
                            tile_position=(32 * g, 0),
                        )
                    nc.scalar.copy(
                        negd2[:, g * GW + base:g * GW + base + PSW], ps[:])
            for m in range(NCH):
                nc.vector.max(cv[:, m * 8:(m + 1) * 8],
                              negd2[:, m * MXCH:(m + 1) * MXCH])
                nc.vector.max_index(ci[:, m * 8:(m + 1) * 8],
                                    cv[:, m * 8:(m + 1) * 8],
                                    negd2[:, m * MXCH:(m + 1) * MXCH])
            nc.sync.dma_start(cv_out[t], cv[:])
            nc.sync.dma_start(ci_out[t], ci[:])

    nc.compile()
    return nc


def _build_prog2():
    nc = bacc.Bacc("TRN2", target_bir_lowering=False, debug=False,
                   num_devices=NCORES)
    # host pre-transposes to partition-major layouts
    geom_in = nc.dram_tensor("geomN", [128, NT, KNN, 18], F32, kind="ExternalInput").ap()
    qgeom_in = nc.dram_tensor("qgeom", [128, NT, 18], F32, kind="ExternalInput").ap()
    vp_in = nc.dram_tensor("vp", [128, NT, KNN], F32, kind="ExternalInput").ap()
    wcross_out = nc.dram_tensor("wcross", [128, NT], F32, kind="ExternalOutput").ap()

    with tile.TileContext(nc) as tc, ExitStack() as ctx:
        pool = ctx.enter_context(tc.tile_pool(name="p", bufs=1))

        TS = NT * KNN
        # small inputs first so the ACT qgr replicate starts immediately;
        # geom as two large half-DMAs on separate HWDGE queues
        nc.sync.dma_start(qg := pool.tile([128, NT, 18], F32, name="qg"),
                          qgeom_in[:])
        nc.scalar.dma_start(vp := pool.tile([128, TS], F32, name="vp"),
                            vp_in[:].rearrange("p t s -> p (t s)"))
        geom = pool.tile([128, TS, 18], F32)
        H = NT // 2
        nc.sync.dma_start(
            geom[:, :H * KNN, :],
            geom_in[:, :H].rearrange("p t s c -> p (t s) c"))
        nc.scalar.dma_start(
            geom[:, H * KNN:, :],
            geom_in[:, H:].rearrange("p t s c -> p (t s) c"))

        # replicate query geometry per neighbor slot (ACT is otherwise idle)
        qgr = pool.tile([128, TS, 18], F32)
        nc.scalar.copy(
            qgr[:].rearrange("p (t s) c -> p t s c", t=NT),
            qg[:].unsqueeze(2).broadcast_to([128, NT, KNN, 18]))

        hit = pool.tile([128, TS, 3, 3], F32)

        def emit(beng, meng, x0, x1):
            """Edge tests for combined (tile, slot) range [x0, x1).
            beng runs the broadcast-AP ops (DVE); meng the unit-stride chain."""
            nx = x1 - x0
            SH = [128, nx, 3, 3]
            xsl = slice(x0, x1)

            def uc(c):   # query edge dir comp c (varies e1)
                return qgr[:, xsl, 9 + c:18:3].unsqueeze(3).broadcast_to(SH)

            def sc(c):   # query edge start comp c
                return qgr[:, xsl, c:9:3].unsqueeze(3).broadcast_to(SH)

            def vc(c):   # neighbor edge dir comp c (varies e2)
                return geom[:, xsl, 9 + c:18:3].unsqueeze(2).broadcast_to(SH)

            def tcp(c):  # neighbor edge start comp c
                return geom[:, xsl, c:9:3].unsqueeze(2).broadcast_to(SH)

            pfx = f"e{x0}"
            m = [pool.tile(SH, F32, name=f"{pfx}_m{i}") for i in range(6)]
            dif = [pool.tile(SH, F32, name=f"{pfx}_d{i}") for i in range(3)]
            cr = [pool.tile(SH, F32, name=f"{pfx}_cr{i}") for i in range(3)]
            BT = beng.tensor_tensor
            MT = meng.tensor_tensor
            for i in range(3):  # cr_i = u_{i+1} * v_{i+2} - u_{i+2} * v_{i+1}
                a, b = (i + 1) % 3, (i + 2) % 3
                BT(m[2 * i][:], uc(a), vc(b), ALU.mult)
                BT(m[2 * i + 1][:], uc(b), vc(a), ALU.mult)
            for c in range(3):
                BT(dif[c][:], tcp(c), sc(c), ALU.subtract)
            for i in range(3):
                MT(cr[i][:], m[2 * i][:], m[2 * i + 1][:], ALU.subtract)

            num = pool.tile(SH, F32, name=f"{pfx}_num")
            den2 = pool.tile(SH, F32, name=f"{pfx}_den2")
            t0 = pool.tile(SH, F32, name=f"{pfx}_t0")
            t1 = pool.tile(SH, F32, name=f"{pfx}_t1")
            MT(num[:], dif[0][:], cr[0][:], ALU.mult)
            MT(den2[:], cr[0][:], cr[0][:], ALU.mult)
            for c in (1, 2):
                MT(t0[:], dif[c][:], cr[c][:], ALU.mult)
                MT(num[:], num[:], t0[:], ALU.add)
                MT(t1[:], cr[c][:], cr[c][:], ALU.mult)
                MT(den2[:], den2[:], t1[:], ALU.add)
            MT(num[:], num[:], num[:], ALU.mult)          # num^2
            meng.tensor_scalar(den2[:], den2[:], float(EPS * EPS), None, ALU.mult)
            h = hit[:, xsl]
            MT(h, num[:], den2[:], ALU.is_lt)             # num^2 < eps^2*|cr|^2
            BT(h, h, vp[:, xsl].unsqueeze(2).unsqueeze(3).broadcast_to(SH),
               ALU.mult)

        emit(nc.vector, nc.vector, 0, TS // 2)
        emit(nc.vector, nc.vector, TS // 2, TS)

        wtile = pool.tile([128, NT], F32)
        nc.vector.tensor_reduce(
            wtile[:], hit[:].rearrange("p (t s) a b -> p t (s a b)", t=NT),
            mybir.AxisListType.X, ALU.add)


        nc.sync.dma_start(wcross_out[:], wtile[:])

    nc.compile()
    return nc


_PROGS = {}


def _get_progs():
    if "p1" not in _PROGS:
        _PROGS["p1"] = _build_prog1()
        _PROGS["p2"] = _build_prog2()
    return _PROGS["p1"], _PROGS["p2"]


def _host_prep(vertices, faces, probabilities):
    V = np.ascontiguousarray(vertices, dtype=np.float32)
    Fc = np.ascontiguousarray(faces).astype(np.int64)
    P = np.ascontiguousarray(probabilities, dtype=np.float32)
    F = Fc.shape[0]

    pos = V[Fc]                                             # [F,3,3]
    bary = (pos[:, 0] + pos[:, 1] + pos[:, 2]) / np.float32(3.0)
    sq = (bary * bary).sum(-1, dtype=np.float32)

    bf = ml_dtypes.bfloat16
    bh = bary.astype(bf).astype(np.float32)
    bl = (bary - bh).astype(bf).astype(np.float32)
    sqh = sq.astype(bf).astype(np.float32)
    sql = (sq - sqh).astype(bf).astype(np.float32)

    rhs = np.zeros((KMM, FP), np.float32)
    rhs[0:3, :F] = (2.0 * bh).T
    rhs[3:6, :F] = (2.0 * bl).T
    rhs[6:9, :F] = (2.0 * bh).T
    rhs[9:12, :F] = (2.0 * bl).T
    rhs[12, :] = -1.0
    rhs[13, :] = -1.0
    rhs[14, :F] = -sqh
    rhs[15, :F] = -sql
    rhs[14, F:] = -1.0e30
    # band b at partitions [32b, 32b+16) holds candidates [b*GW, (b+1)*GW)
    rhs_bf = rhs.astype(bf)
    rhs_b = np.zeros((128, GW), bf)
    for b in range(NGRP):
        rhs_b[32 * b:32 * b + KMM] = rhs_bf[:, b * GW:(b + 1) * GW]

    lhsT = np.zeros((KMM, FP), np.float32)
    lhsT[0:3, :F] = bh.T
    lhsT[3:6, :F] = bh.T
    lhsT[6:9, :F] = bl.T
    lhsT[9:12, :F] = bl.T
    lhsT[12, :F] = sqh
    lhsT[13, :F] = sql
    lhsT[14, :] = 1.0
    lhsT[15, :] = 1.0
    lhsT_bf = lhsT.astype(bf)
    lhsT_b = np.zeros((128, FP), bf)
    for b in range(NGRP):
        lhsT_b[32 * b:32 * b + KMM] = lhsT_bf

    starts = pos[:, [0, 0, 1], :].reshape(F, 9)
    dirs = (pos[:, [1, 2, 2], :] - pos[:, [0, 0, 1], :]).reshape(F, 9)
    geo = np.zeros((FP, 18), np.float32)
    geo[:F, 0:9] = starts
    geo[:F, 9:18] = dirs

    probs_pad = np.zeros(FP, np.float32)
    probs_pad[:F] = P

    in1 = []
    for c in range(NCORES):
        lo, hi = c * NR, (c + 1) * NR
        in1.append({
            "lhsT": np.ascontiguousarray(lhsT_b[:, lo:hi]),
            "rhs": rhs_b,
        })
    aux = dict(F=F, geo=geo, probs_pad=probs_pad,
               bary=bary, sq=sq, bh=bh, bl=bl, sqh=sqh, sql=sql)
    return in1, aux


def _exact_rows_negd2(rows, aux):
    """Replicate the device -d2 rows in f32 (bf16-split products, f32 sums)."""
    bh, bl, sqh, sql = aux["bh"], aux["bl"], aux["sqh"], aux["sql"]
    F = aux["F"]
    rows = np.asarray(rows)
    live = rows < F                     # pad query rows have all-zero terms
    rc = np.where(live, rows, 0)
    S = len(rows)
    acc = np.zeros((S, FP), np.float32)
    for qp, cp in ((bh, bh), (bl, bh), (bh, bl), (bl, bl)):
        acc[:, :F] += (2 * qp[rc] * live[:, None]) @ cp.T
    acc[:, :F] -= ((sqh[rc] + sql[rc]) * live)[:, None]
    acc[:, :F] -= (sqh + sql)[None, :F]
    acc[:, F:] = -1.0e30
    return acc


def _host_merge(res1, aux):
    """Exact top-16 merge of per-chunk top-8s; returns nbr [FP, 16]."""
    vals = np.empty((FP, NC8), np.float32)
    lidx = np.empty((FP, NC8), np.uint16)
    for c in range(NCORES):
        vals[c * NR:(c + 1) * NR] = \
            np.asarray(res1.results[c]["cv"]).reshape(NR, NC8)
        lidx[c * NR:(c + 1) * NR] = \
            np.asarray(res1.results[c]["ci"]).reshape(NR, NC8)
    gidx = lidx.astype(np.int64) + \
        (np.arange(NC8, dtype=np.int64) // 8 * MXCH)[None, :]

    part = np.argpartition(-vals, KNN, axis=1)[:, :KNN]
    pv = np.take_along_axis(vals, part, axis=1)
    pg = np.take_along_axis(gidx, part, axis=1)
    order = np.lexsort((pg, -pv), axis=1)
    nbr = np.take_along_axis(pg, order, axis=1)             # [FP, 16]
    nv = np.take_along_axis(pv, order, axis=1)

    # truncation fallback: a chunk whose reported 8 values are all >= our
    # 16th could hide an unreported 9th that belongs in the top-16.
    F = aux["F"]
    v16 = nv[:, KNN - 1]
    chunk_min = vals[:, 7::8]                               # 8th value of each chunk
    suspect = np.nonzero((chunk_min >= v16[:, None]).any(1)
                         & (np.arange(FP) < F))[0]
    if suspect.size:
        negd2 = _exact_rows_negd2(suspect, aux)
        prt = np.argpartition(-negd2, KNN, axis=1)[:, :KNN]
        pvv = np.take_along_axis(negd2, prt, axis=1)
        o = np.lexsort((prt, -pvv), axis=1)
        nbr[suspect] = np.take_along_axis(prt, o, axis=1)
    return nbr


def _run(vertices, faces, probabilities, trace=False, **kw):
    p1, p2 = _get_progs()
    in1, aux = _host_prep(vertices, faces, probabilities)
    res1 = run_bass_kernel_spmd(p1, in1, list(range(NCORES)), trace=trace, **kw)
    nbr = _host_merge(res1, aux)                            # [FP, 16]
    F = aux["F"]

    geo = aux["geo"]
    geomN = geo[nbr]                                        # [FP, 16, 18]
    vp = (nbr != np.arange(FP)[:, None]).astype(np.float32) \
        * aux["probs_pad"][:, None]                         # [FP, 16]

    in2 = []
    for c in range(NCORES):
        lo, hi = c * NR, (c + 1) * NR
        in2.append({
            "geomN": np.ascontiguousarray(
                geomN[lo:hi].reshape(NT, 128, KNN, 18).transpose(1, 0, 2, 3)),
            "qgeom": np.ascontiguousarray(
                geo[lo:hi].reshape(NT, 128, 18).transpose(1, 0, 2)),
            "vp": np.ascontiguousarray(
                vp[lo:hi].reshape(NT, 128, KNN).transpose(1, 0, 2)),
        })
    res2 = run_bass_kernel_spmd(p2, in2, list(range(NCORES)), trace=trace, **kw)

    total = np.float64(0.0)
    for c in range(NCORES):
        total += np.asarray(res2.results[c]["wcross"], dtype=np.float64).sum()
    loss = np.float32(total / F)
    return loss, res1, res2, nbr


def run_device(vertices, faces, probabilities, trace=False, **kw):
    loss, res1, res2, _ = _run(vertices, faces, probabilities, trace=trace, **kw)
    return loss, (res1, res2)


def kernel(vertices, faces, probabilities):
    loss, *_ = _run(vertices, faces, probabilities)
    return np.array(loss, dtype=np.float32)



# revision 28
# speedup vs baseline: 2.2146x; 2.2146x over previous
"""EdgeCrossingsLoss Trainium2 kernel (8-core SPMD, data-parallel over query faces).

Two device launches (no usable on-device gather in this runtime; the host does
the small index-merge + geometry gather between the launches):

prog1 (per core, 1280 query rows = 10 tiles of 128):
  PE:  -d2[q, c] for all 10240 candidates via a K=16 bf16 hi/lo-split matmul
       (bf16 products are exact, accumulated in f32 PSUM). rhs sits in four
       16-partition bands at base partitions 0/32/64/96.
  Reduction: instead of two full f32 DVE scans (max8 + max_index, the old
       bottleneck), a 4-level binary max tree computes window-16 maxima of
       -d2 in bf16. Level 1 (f32 PSUM -> bf16 SBUF) is split across engines:
       ACT copies some PSUM pieces to bf16 (DVE then pair-maxes them at 2x
       bf16 rate), GPSIMD pair-maxes other pieces straight out of PSUM, DVE
       takes the remainder. Levels 2-4 run on DVE in bf16 (2x mode). The full
       [128, 640] window-max tile is DMA'd out; no on-device top-k at all.

host: picks the top-24 windows per row from the 640 bf16 window maxima,
      resolves all 24*16 member candidates exactly (f32), takes the exact
      top-16 with the jax tie-break. Rows where the 25th-best window max
      could hide a true top-16 member (value margin covering bf16 rounding)
      are recomputed exactly (vectorized, ~a few % of rows). Gathers the 16
      neighbor faces' edge geometry; folds probabilities and the
      self-neighbor mask into per-(row, slot) weights.

prog2 (per core): all 1280x16 3x3 line-line crossing tests, engine-split:
      DVE runs the broadcast-AP ops + part of the unit-stride chain, GPSIMD
      runs compares/reduction + part of the chain, ACT squares (cr^2, num^2)
      and the eps^2 scaling. hit = num^2 < EPS^2*|cross|^2 (den=0 / NaN cases
      fall out correctly), weight-masked per slot and reduced per (tile,slot).

Host sums the 8 per-core partials and divides by num_faces.
"""
import os
import numpy as np
import ml_dtypes
from contextlib import ExitStack

import concourse.bass as bass
import concourse.tile as tile
import concourse.bacc as bacc
from concourse import mybir
from concourse.bass_utils import run_bass_kernel_spmd

F32 = mybir.dt.float32
BF16 = mybir.dt.bfloat16
U16 = mybir.dt.uint16

NCORES = 8
KNN = 16
EPS = 1e-5
FP = 10240            # padded candidate count
NR = FP // NCORES     # 1280 rows per core
NT = NR // 128        # 10 tiles of 128 rows
KMM = 16              # matmul contraction rows (bf16 hi/lo split)
NGRP = 4              # rhs partition bands (at partitions 0/32/64/96)
GW = FP // NGRP       # 2560 candidates per band
PIECE = 1024          # PSUM piece width (f32, exactly 2 banks -> 4-deep ring)
NPIECE = FP // PIECE  # 10 pieces per tile
MMCH = 512            # matmul N per instruction (one PSUM bank)
NWIN = FP // 4        # 2560 window-4 maxima per row
MWIN = 32             # host resolves top-32 windows per row

ALU = mybir.AluOpType
AFT = mybir.ActivationFunctionType


def _build_prog1():
    nc = bacc.Bacc("TRN2", target_bir_lowering=False, debug=False,
                   num_devices=NCORES)
    # band b occupies partitions [32b, 32b+16); lhsT replicated into each band
    lhsT_in = nc.dram_tensor("lhsT", [128, NR], BF16, kind="ExternalInput").ap()
    rhs_in = nc.dram_tensor("rhs", [128, GW], BF16, kind="ExternalInput").ap()
    wm_out = nc.dram_tensor("wm", [NT, 128, NWIN], BF16,
                            kind="ExternalOutput").ap()

    with tile.TileContext(nc) as tc, ExitStack() as ctx:
        const_pool = ctx.enter_context(tc.tile_pool(name="const", bufs=1))
        psum_pool = ctx.enter_context(tc.tile_pool(name="psum", bufs=4,
                                                   space="PSUM"))
        ab_pool = ctx.enter_context(tc.tile_pool(name="ab", bufs=2))
        l1_pool = ctx.enter_context(tc.tile_pool(name="l1", bufs=2))
        wm_pool = ctx.enter_context(tc.tile_pool(name="wmp", bufs=2))

        lhsT_sb = const_pool.tile([128, NR], BF16)
        nc.sync.dma_start(lhsT_sb[:], lhsT_in[:])
        rhs_sb = const_pool.tile([128, GW], BF16)
        for j in range(4):   # column chunks on two queues: matmuls start early
            eng = (nc.scalar, nc.sync)[j % 2]
            eng.dma_start(rhs_sb[:, j * (GW // 4):(j + 1) * (GW // 4)],
                          rhs_in[:, j * (GW // 4):(j + 1) * (GW // 4)])

        # PSUM-read rules on TRN2: GPSIMD may not touch PSUM at all, and a
        # TensorTensor may read at most ONE operand from PSUM. GPSIMD also
        # lacks max/min/compare ops in this runtime (add/sub/mult and
        # tensor_scalar max only). So: DVE tensor_reduce (single PSUM input,
        # strided [128, 256, 4] view) drains AND window-4-maxes 4 pieces per
        # tile in one op each; ACT copies the other 6 pieces to bf16 SBUF
        # where the 2-level contiguous-half pair-max tree runs on DVE (bf16
        # 2x) or on GPSIMD via the 3-op identity max(x,y) = y + relu(x-y).
        # Windows are t-major: window w of piece p holds candidates
        # 1024*p + (w%256) + 256*t, t = 0..3.
        KINDS = "ADADAADADA"                  # 6 ACT pieces, 4 DVE pieces
        TREES = ["dddddd", "dddddd"]          # per-tile tree engines (A order)

        def pool_pair_max(dst, x, y, tmp):
            nc.gpsimd.tensor_tensor(tmp, x, y, ALU.subtract)
            nc.gpsimd.tensor_scalar(tmp, tmp, 0.0, None, ALU.max)
            nc.gpsimd.tensor_tensor(dst, y, tmp, ALU.add)

        for t in range(NT):
            trees = TREES[t % 2]
            abuf = ab_pool.tile([128, 6, PIECE], BF16, tag="ab")
            wm = wm_pool.tile([128, NPIECE, PIECE // 4], BF16, tag="wm")
            na = 0
            for p in range(NPIECE):
                ps = psum_pool.tile([128, PIECE], F32, tag="ps")
                for c0 in range(0, PIECE, MMCH):
                    gcol = p * PIECE + c0      # global candidate column
                    g = gcol // GW             # band
                    off = gcol - g * GW
                    nc.tensor.matmul(
                        ps[:, c0:c0 + MMCH],
                        lhsT=lhsT_sb[32 * g:32 * g + KMM,
                                     t * 128:(t + 1) * 128],
                        rhs=rhs_sb[32 * g:32 * g + KMM, off:off + MMCH],
                        start=True, stop=True,
                        tile_position=(32 * g, 0),
                    )
                Q = PIECE // 4
                if KINDS[p] == "A":
                    nc.scalar.copy(abuf[:, na, :], ps[:])
                    ab = abuf[:, na, :]
                    if trees[na] == "d":
                        l1 = l1_pool.tile([128, 2, Q], BF16, tag="l1")
                        nc.vector.tensor_tensor(
                            l1[:], ab[0:128, 0:2 * Q].rearrange(
                                "p (l w) -> p l w", l=2),
                            ab[0:128, 2 * Q:4 * Q].rearrange(
                                "p (l w) -> p l w", l=2), ALU.max)
                        nc.vector.tensor_tensor(wm[:, p, :], l1[:, 0, :],
                                                l1[:, 1, :], ALU.max)
                    else:
                        l1 = l1_pool.tile([128, 2, Q], BF16, tag="l1p")
                        tmp = l1_pool.tile([128, 2, Q], BF16, tag="tmpp")
                        nc.gpsimd.tensor_tensor(
                            tmp[:], ab[0:128, 0:2 * Q].rearrange(
                                "p (l w) -> p l w", l=2),
                            ab[0:128, 2 * Q:4 * Q].rearrange(
                                "p (l w) -> p l w", l=2), ALU.subtract)
                        nc.gpsimd.tensor_scalar(tmp[:], tmp[:], 0.0, None,
                                                ALU.max)
                        nc.gpsimd.tensor_tensor(
                            l1[:], ab[0:128, 2 * Q:4 * Q].rearrange(
                                "p (l w) -> p l w", l=2), tmp[:], ALU.add)
                        pool_pair_max(wm[:, p, :], l1[:, 0, :], l1[:, 1, :],
                                      tmp[:, 0, :])
                    na += 1
                else:
                    nc.vector.tensor_reduce(
                        wm[:, p, :],
                        ps[:].rearrange("p (t w) -> p w t", t=4),
                        mybir.AxisListType.X, ALU.max)
            eng = (nc.sync, nc.scalar)[t % 2]
            eng.dma_start(wm_out[t], wm[:].rearrange("p a b -> p (a b)"))

    nc.compile()
    return nc


def _build_prog2():
    nc = bacc.Bacc("TRN2", target_bir_lowering=False, debug=False,
                   num_devices=NCORES)
    # host pre-transposes to partition-major layouts
    geom_in = nc.dram_tensor("geomN", [128, NT, KNN, 18], F32,
                             kind="ExternalInput").ap()
    qgeom_in = nc.dram_tensor("qgeom", [128, NT, 18], F32,
                              kind="ExternalInput").ap()
    vp_in = nc.dram_tensor("vp", [128, NT, KNN], F32, kind="ExternalInput").ap()
    hw_out = nc.dram_tensor("hw", [128, NT * KNN], F32,
                            kind="ExternalOutput").ap()

    with tile.TileContext(nc) as tc, ExitStack() as ctx:
        pool = ctx.enter_context(tc.tile_pool(name="p", bufs=1))

        TS = NT * KNN
        # small inputs first so the ACT qgr replicate starts immediately;
        # geom as two large half-DMAs on separate HWDGE queues
        nc.sync.dma_start(qg := pool.tile([128, NT, 18], F32, name="qg"),
                          qgeom_in[:])
        nc.scalar.dma_start(vp := pool.tile([128, TS], F32, name="vp"),
                            vp_in[:].rearrange("p t s -> p (t s)"))
        geom = pool.tile([128, TS, 18], F32)
        H = NT // 2
        nc.sync.dma_start(
            geom[:, :H * KNN, :],
            geom_in[:, :H].rearrange("p t s c -> p (t s) c"))
        nc.scalar.dma_start(
            geom[:, H * KNN:, :],
            geom_in[:, H:].rearrange("p t s c -> p (t s) c"))

        # replicate query geometry per neighbor slot (on ACT)
        qgr = pool.tile([128, TS, 18], F32)
        nc.scalar.copy(
            qgr[:].rearrange("p (t s) c -> p t s c", t=NT),
            qg[:].unsqueeze(2).broadcast_to([128, NT, KNN, 18]))

        hwsum = pool.tile([128, TS], F32)

        def emit_a(x0, x1):
            """Stage A: geometry products for (tile, slot) range [x0, x1)."""
            nx = x1 - x0
            SH = [128, nx, 3, 3]
            xsl = slice(x0, x1)

            def uc(c):   # query edge dir comp c (varies e1)
                return qgr[:, xsl, 9 + c:18:3].unsqueeze(3).broadcast_to(SH)

            def sc(c):   # query edge start comp c
                return qgr[:, xsl, c:9:3].unsqueeze(3).broadcast_to(SH)

            def vc(c):   # neighbor edge dir comp c (varies e2)
                return geom[:, xsl, 9 + c:18:3].unsqueeze(2).broadcast_to(SH)

            def tcp(c):  # neighbor edge start comp c
                return geom[:, xsl, c:9:3].unsqueeze(2).broadcast_to(SH)

            pfx = f"e{x0}"
            m = [pool.tile(SH, F32, name=f"{pfx}_m{i}") for i in range(6)]
            dif = [pool.tile(SH, F32, name=f"{pfx}_d{i}") for i in range(3)]
            cr = [pool.tile(SH, F32, name=f"{pfx}_cr{i}") for i in range(3)]
            DV = nc.vector.tensor_tensor
            GP = nc.gpsimd.tensor_tensor
            # broadcast-AP ops: DVE only (GPSIMD rejects broadcast APs)
            for i in range(3):  # cr_i = u_{i+1} * v_{i+2} - u_{i+2} * v_{i+1}
                a, b = (i + 1) % 3, (i + 2) % 3
                DV(m[2 * i][:], uc(a), vc(b), ALU.mult)
                DV(m[2 * i + 1][:], uc(b), vc(a), ALU.mult)
            for c in range(3):
                DV(dif[c][:], tcp(c), sc(c), ALU.subtract)
            # unit-stride chain, split DVE / GPSIMD
            DV(cr[0][:], m[0][:], m[1][:], ALU.subtract)
            GP(cr[1][:], m[2][:], m[3][:], ALU.subtract)
            GP(cr[2][:], m[4][:], m[5][:], ALU.subtract)

            num = pool.tile(SH, F32, name=f"{pfx}_num")
            t0 = pool.tile(SH, F32, name=f"{pfx}_t0")
            t1 = pool.tile(SH, F32, name=f"{pfx}_t1")
            DV(num[:], dif[0][:], cr[0][:], ALU.mult)
            GP(t0[:], dif[1][:], cr[1][:], ALU.mult)
            GP(t1[:], dif[2][:], cr[2][:], ALU.mult)
            DV(num[:], num[:], t0[:], ALU.add)
            GP(num[:], num[:], t1[:], ALU.add)


            # den2 = cr0^2 + cr1^2 + cr2^2: squares on ACT
            s0 = pool.tile(SH, F32, name=f"{pfx}_s0")
            s1 = pool.tile(SH, F32, name=f"{pfx}_s1")
            s2 = pool.tile(SH, F32, name=f"{pfx}_s2")
            nc.scalar.activation(s0[:], cr[0][:], AFT.Square)
            nc.scalar.activation(s1[:], cr[1][:], AFT.Square)
            nc.scalar.activation(s2[:], cr[2][:], AFT.Square)
            GP(s0[:], s0[:], s1[:], ALU.add)
            GP(s0[:], s0[:], s2[:], ALU.add)
            # (num/eps)^2 on ACT (scale folded into the square): the hit test
            # num^2 < eps^2*den2 becomes (num/eps)^2 < den2 directly.
            num2 = pool.tile(SH, F32, name=f"{pfx}_n2")
            nc.scalar.activation(num2[:], num[:], AFT.Square,
                                 scale=float(1.0 / EPS))
            return num2, s0

        def emit_b(x0, x1, num2, s0):
            """Stage B: hit test + weighted per-slot reduction."""
            nx = x1 - x0
            SH = [128, nx, 3, 3]
            xsl = slice(x0, x1)
            pfx = f"e{x0}"
            DV = nc.vector.tensor_tensor
            hit = pool.tile(SH, F32, name=f"{pfx}_hit")
            DV(hit[:], num2[:], s0[:], ALU.is_lt)    # GPSIMD lacks compares
            hred = pool.tile([128, nx], F32, name=f"{pfx}_hr")
            nc.vector.tensor_reduce(
                hred[:], hit[:].rearrange("p s a b -> p s (a b)"),
                mybir.AxisListType.X, ALU.add)
            DV(hwsum[:, xsl], hred[:], vp[:, xsl], ALU.mult)

        NQ = 4   # quarters, software-pipelined: stage B of quarter q is
        # emitted after stage A of quarter q+1 so engine queues never block
        pend = None
        for q in range(NQ):
            x0, x1 = q * TS // NQ, (q + 1) * TS // NQ
            cur = (x0, x1) + emit_a(x0, x1)
            if pend is not None:
                emit_b(*pend)
            pend = cur
        emit_b(*pend)

        nc.sync.dma_start(hw_out[:], hwsum[:])

    nc.compile()
    return nc


_PROGS = {}


def _get_progs():
    if "p1" not in _PROGS:
        _PROGS["p1"] = _build_prog1()
        _PROGS["p2"] = _build_prog2()
    return _PROGS["p1"], _PROGS["p2"]


def _host_prep(vertices, faces, probabilities):
    V = np.ascontiguousarray(vertices, dtype=np.float32)
    Fc = np.ascontiguousarray(faces).astype(np.int64)
    P = np.ascontiguousarray(probabilities, dtype=np.float32)
    F = Fc.shape[0]

    pos = V[Fc]                                             # [F,3,3]
    bary = (pos[:, 0] + pos[:, 1] + pos[:, 2]) / np.float32(3.0)
    sq = (bary * bary).sum(-1, dtype=np.float32)

    bf = ml_dtypes.bfloat16
    bh = bary.astype(bf).astype(np.float32)
    bl = (bary - bh).astype(bf).astype(np.float32)
    sqh = sq.astype(bf).astype(np.float32)
    sql = (sq - sqh).astype(bf).astype(np.float32)

    rhs = np.zeros((KMM, FP), np.float32)
    rhs[0:3, :F] = (2.0 * bh).T
    rhs[3:6, :F] = (2.0 * bl).T
    rhs[6:9, :F] = (2.0 * bh).T
    rhs[9:12, :F] = (2.0 * bl).T
    rhs[12, :] = -1.0
    rhs[13, :] = -1.0
    rhs[14, :F] = -sqh
    rhs[15, :F] = -sql
    rhs[14, F:] = -1.0e30
    # band b at partitions [32b, 32b+16) holds candidates [b*GW, (b+1)*GW)
    rhs_bf = rhs.astype(bf)
    rhs_b = np.zeros((128, GW), bf)
    for b in range(NGRP):
        rhs_b[32 * b:32 * b + KMM] = rhs_bf[:, b * GW:(b + 1) * GW]

    lhsT = np.zeros((KMM, FP), np.float32)
    lhsT[0:3, :F] = bh.T
    lhsT[3:6, :F] = bh.T
    lhsT[6:9, :F] = bl.T
    lhsT[9:12, :F] = bl.T
    lhsT[12, :F] = sqh
    lhsT[13, :F] = sql
    lhsT[14, :] = 1.0
    lhsT[15, :] = 1.0
    lhsT_bf = lhsT.astype(bf)
    lhsT_b = np.zeros((128, FP), bf)
    for b in range(NGRP):
        lhsT_b[32 * b:32 * b + KMM] = lhsT_bf

    starts = pos[:, [0, 0, 1], :].reshape(F, 9)
    dirs = (pos[:, [1, 2, 2], :] - pos[:, [0, 0, 1], :]).reshape(F, 9)
    geo = np.zeros((FP, 18), np.float32)
    geo[:F, 0:9] = starts
    geo[:F, 9:18] = dirs

    probs_pad = np.zeros(FP, np.float32)
    probs_pad[:F] = P

    bary_pad = np.zeros((FP, 3), np.float32)
    bary_pad[:F] = bary
    sq_pad = np.zeros(FP, np.float32)
    sq_pad[:F] = sq

    in1 = []
    for c in range(NCORES):
        lo, hi = c * NR, (c + 1) * NR
        in1.append({
            "lhsT": np.ascontiguousarray(lhsT_b[:, lo:hi]),
            "rhs": rhs_b,
        })
    aux = dict(F=F, geo=geo, probs_pad=probs_pad,
               bary=bary, sq=sq, bh=bh, bl=bl, sqh=sqh, sql=sql,
               bary_pad=bary_pad, sq_pad=sq_pad)
    return in1, aux


def _exact_rows_negd2(rows, aux):
    """Replicate the device -d2 rows in f32 (bf16-split products, f32 sums)."""
    bh, bl, sqh, sql = aux["bh"], aux["bl"], aux["sqh"], aux["sql"]
    F = aux["F"]
    rows = np.asarray(rows)
    live = rows < F                     # pad query rows have all-zero terms
    rc = np.where(live, rows, 0)
    S = len(rows)
    acc = np.zeros((S, FP), np.float32)
    for qp, cp in ((bh, bh), (bl, bh), (bh, bl), (bl, bl)):
        acc[:, :F] += (2 * qp[rc] * live[:, None]) @ cp.T
    acc[:, :F] -= ((sqh[rc] + sql[rc]) * live)[:, None]
    acc[:, :F] -= (sqh + sql)[None, :F]
    acc[:, F:] = -1.0e30
    return acc


def _host_merge(res1, aux):
    """Window-max merge: resolve the top-MWIN windows per row exactly;
    value-margin fallback to a full exact recompute. Returns nbr [FP, 16]."""
    F = aux["F"]
    wm = np.empty((FP, NWIN), np.float32)
    for c in range(NCORES):
        wm[c * NR:(c + 1) * NR] = np.asarray(
            res1.results[c]["wm"]).astype(np.float32).reshape(NR, NWIN)

    part = np.argpartition(-wm, MWIN, axis=1)
    topw = part[:, :MWIN]                                   # [FP, MWIN]
    w25 = np.take_along_axis(
        wm, part[:, MWIN:MWIN + 1], axis=1)[:, 0]           # (MWIN+1)-th value
    # rank within the partition isn't sorted; w25 must be max of the rest
    rest = np.take_along_axis(wm, part[:, MWIN:], axis=1)
    w25 = rest.max(axis=1)

    # window W (0..2559) of piece p=W//256 holds candidates
    # 1024*p + (W%256) + 256*t, t=0..3 (t-major grouping)
    k = topw // 256
    w = topw % 256
    cand = (1024 * k + w)[:, :, None] + \
        (256 * np.arange(4, dtype=np.int64))[None, None, :]
    cand = cand.reshape(FP, MWIN * 4)                       # [FP, 128]

    bary_pad, sq_pad = aux["bary_pad"], aux["sq_pad"]
    dot = np.einsum("rd,rmd->rm", bary_pad, bary_pad[cand], optimize=True)
    v = 2.0 * dot - sq_pad[:, None] - sq_pad[cand]
    v = v.astype(np.float32)
    v[cand >= F] = -1.0e30

    prt = np.argpartition(-v, KNN, axis=1)[:, :KNN]
    pv = np.take_along_axis(v, prt, axis=1)
    pc = np.take_along_axis(cand, prt, axis=1)
    order = np.lexsort((pc, -pv), axis=1)
    nbr = np.take_along_axis(pc, order, axis=1)             # [FP, 16]
    nv = np.take_along_axis(pv, order, axis=1)
    v16 = nv[:, KNN - 1]

    # fallback: the (MWIN+1)-th window max (plus bf16 + recompute margin)
    # could hide an unresolved true top-16 member.
    margin = np.abs(w25) * (2.0 ** -7) + 1e-5
    suspect = np.nonzero((w25 + margin >= v16) & (np.arange(FP) < F))[0]
    if suspect.size:
        negd2 = _exact_rows_negd2(suspect, aux)
        sp = np.argpartition(-negd2, KNN, axis=1)[:, :KNN]
        spv = np.take_along_axis(negd2, sp, axis=1)
        o = np.lexsort((sp, -spv), axis=1)
        nbr[suspect] = np.take_along_axis(sp, o, axis=1)
    return nbr


def _run(vertices, faces, probabilities, trace=False, **kw):
    p1, p2 = _get_progs()
    in1, aux = _host_prep(vertices, faces, probabilities)
    res1 = run_bass_kernel_spmd(p1, in1, list(range(NCORES)), trace=trace, **kw)
    nbr = _host_merge(res1, aux)                            # [FP, 16]
    F = aux["F"]

    geo = aux["geo"]
    geomN = geo[nbr]                                        # [FP, 16, 18]
    vp = (nbr != np.arange(FP)[:, None]).astype(np.float32) \
        * aux["probs_pad"][:, None]                         # [FP, 16]

    in2 = []
    for c in range(NCORES):
        lo, hi = c * NR, (c + 1) * NR
        in2.append({
            "geomN": np.ascontiguousarray(
                geomN[lo:hi].reshape(NT, 128, KNN, 18).transpose(1, 0, 2, 3)),
            "qgeom": np.ascontiguousarray(
                geo[lo:hi].reshape(NT, 128, 18).transpose(1, 0, 2)),
            "vp": np.ascontiguousarray(
                vp[lo:hi].reshape(NT, 128, KNN).transpose(1, 0, 2)),
        })
    res2 = run_bass_kernel_spmd(p2, in2, list(range(NCORES)), trace=trace, **kw)

    total = np.float64(0.0)
    for c in range(NCORES):
        total += np.asarray(res2.results[c]["hw"], dtype=np.float64).sum()
    loss = np.float32(total / F)
    return loss, res1, res2, nbr


def run_device(vertices, faces, probabilities, trace=False, **kw):
    loss, res1, res2, _ = _run(vertices, faces, probabilities, trace=trace, **kw)
    return loss, (res1, res2)


def kernel(vertices, faces, probabilities):
    loss, *_ = _run(vertices, faces, probabilities)
    return np.array(loss, dtype=np.float32)


# revision 35
# speedup vs baseline: 2.2179x; 1.0015x over previous
"""EdgeCrossingsLoss Trainium2 kernel (8-core SPMD, data-parallel over query faces).

Two device launches (no usable on-device gather in this runtime; the host does
the small index-merge + geometry gather between the launches):

prog1 (per core, 1280 query rows = 10 tiles of 128):
  PE:  -d2[q, c] for all 10240 candidates via a K=16 bf16 hi/lo-split matmul
       (bf16 products are exact, accumulated in f32 PSUM). rhs sits in four
       16-partition bands at base partitions 0/32/64/96.
  Reduction: instead of two full f32 DVE scans (max8 + max_index, the old
       bottleneck), a 4-level binary max tree computes window-16 maxima of
       -d2 in bf16. Level 1 (f32 PSUM -> bf16 SBUF) is split across engines:
       ACT copies some PSUM pieces to bf16 (DVE then pair-maxes them at 2x
       bf16 rate), GPSIMD pair-maxes other pieces straight out of PSUM, DVE
       takes the remainder. Levels 2-4 run on DVE in bf16 (2x mode). The full
       [128, 640] window-max tile is DMA'd out; no on-device top-k at all.

host: picks the top-24 windows per row from the 640 bf16 window maxima,
      resolves all 24*16 member candidates exactly (f32), takes the exact
      top-16 with the jax tie-break. Rows where the 25th-best window max
      could hide a true top-16 member (value margin covering bf16 rounding)
      are recomputed exactly (vectorized, ~a few % of rows). Gathers the 16
      neighbor faces' edge geometry; folds probabilities and the
      self-neighbor mask into per-(row, slot) weights.

prog2 (per core): all 1280x16 3x3 line-line crossing tests, engine-split:
      DVE runs the broadcast-AP ops + part of the unit-stride chain, GPSIMD
      runs compares/reduction + part of the chain, ACT squares (cr^2, num^2)
      and the eps^2 scaling. hit = num^2 < EPS^2*|cross|^2 (den=0 / NaN cases
      fall out correctly), weight-masked per slot and reduced per (tile,slot).

Host sums the 8 per-core partials and divides by num_faces.
"""
import os
import numpy as np
import ml_dtypes
from contextlib import ExitStack

import concourse.bass as bass
import concourse.tile as tile
import concourse.bacc as bacc
from concourse import mybir
from concourse.bass_utils import run_bass_kernel_spmd

F32 = mybir.dt.float32
BF16 = mybir.dt.bfloat16
U16 = mybir.dt.uint16

NCORES = 8
KNN = 16
EPS = 1e-5
FP = 10240            # padded candidate count
NR = FP // NCORES     # 1280 rows per core
NT = NR // 128        # 10 tiles of 128 rows
KMM = 16              # matmul contraction rows (bf16 hi/lo split)
NGRP = 4              # rhs partition bands (at partitions 0/32/64/96)
GW = FP // NGRP       # 2560 candidates per band
PIECE = 1024          # PSUM piece width (f32, exactly 2 banks -> 4-deep ring)
NPIECE = FP // PIECE  # 10 pieces per tile
MMCH = 512            # matmul N per instruction (one PSUM bank)
NWIN = FP // 4        # 2560 window-4 maxima per row
MWIN = 32             # host resolves top-32 windows per row

ALU = mybir.AluOpType
AFT = mybir.ActivationFunctionType


def _build_prog1():
    nc = bacc.Bacc("TRN2", target_bir_lowering=False, debug=False,
                   num_devices=NCORES)
    # band b occupies partitions [32b, 32b+16); lhsT replicated into each band
    lhsT_in = nc.dram_tensor("lhsT", [128, NR], BF16, kind="ExternalInput").ap()
    rhs_in = nc.dram_tensor("rhs", [128, GW], BF16, kind="ExternalInput").ap()
    wm_out = nc.dram_tensor("wm", [NT, 128, NWIN], BF16,
                            kind="ExternalOutput").ap()

    with tile.TileContext(nc) as tc, ExitStack() as ctx:
        const_pool = ctx.enter_context(tc.tile_pool(name="const", bufs=1))
        psum_pool = ctx.enter_context(tc.tile_pool(name="psum", bufs=4,
                                                   space="PSUM"))
        ab_pool = ctx.enter_context(tc.tile_pool(name="ab", bufs=3))
        l1_pool = ctx.enter_context(tc.tile_pool(name="l1", bufs=6))
        wm_pool = ctx.enter_context(tc.tile_pool(name="wmp", bufs=3))

        lhsT_sb = const_pool.tile([128, NR], BF16)
        nc.sync.dma_start(lhsT_sb[:], lhsT_in[:])
        rhs_sb = const_pool.tile([128, GW], BF16)
        for j in range(4):   # column chunks on two queues: matmuls start early
            eng = (nc.scalar, nc.sync)[j % 2]
            eng.dma_start(rhs_sb[:, j * (GW // 4):(j + 1) * (GW // 4)],
                          rhs_in[:, j * (GW // 4):(j + 1) * (GW // 4)])

        # PSUM-read rules on TRN2: GPSIMD may not touch PSUM at all, and a
        # TensorTensor may read at most ONE operand from PSUM. GPSIMD also
        # lacks max/min/compare ops in this runtime (add/sub/mult and
        # tensor_scalar max only). So: DVE tensor_reduce (single PSUM input,
        # strided [128, 256, 4] view) drains AND window-4-maxes 4 pieces per
        # tile in one op each; ACT copies the other 6 pieces to bf16 SBUF
        # where the 2-level contiguous-half pair-max tree runs on DVE (bf16
        # 2x) or on GPSIMD via the 3-op identity max(x,y) = y + relu(x-y).
        # Windows are t-major: window w of piece p holds candidates
        # 1024*p + (w%256) + 256*t, t = 0..3.
        KINDS = "ADADAADADA"                  # 6 ACT pieces, 4 DVE pieces
        # tree engine per ACT piece: d = both levels DVE; s = level 1 on DVE,
        # level 2 on GPSIMD via the exact 3-op identity max(x,y)=y+relu(x-y)
        TREES = ["ddddds", "ddddds"]

        def pool_pair_max(dst, x, y, tmp):
            nc.gpsimd.tensor_tensor(tmp, x, y, ALU.subtract)
            nc.gpsimd.tensor_scalar(tmp, tmp, 0.0, None, ALU.max)
            nc.gpsimd.tensor_tensor(dst, y, tmp, ALU.add)

        for t in range(NT):
            trees = TREES[t % 2]
            abuf = ab_pool.tile([128, 6, PIECE], BF16, tag="ab")
            wm = wm_pool.tile([128, NPIECE, PIECE // 4], BF16, tag="wm")
            na = 0
            for p in range(NPIECE):
                ps = psum_pool.tile([128, PIECE], F32, tag="ps")
                for c0 in range(0, PIECE, MMCH):
                    gcol = p * PIECE + c0      # global candidate column
                    g = gcol // GW             # band
                    off = gcol - g * GW
                    nc.tensor.matmul(
                        ps[:, c0:c0 + MMCH],
                        lhsT=lhsT_sb[32 * g:32 * g + KMM,
                                     t * 128:(t + 1) * 128],
                        rhs=rhs_sb[32 * g:32 * g + KMM, off:off + MMCH],
                        start=True, stop=True,
                        tile_position=(32 * g, 0),
                    )
                Q = PIECE // 4
                if KINDS[p] == "A":
                    nc.scalar.copy(abuf[:, na, :], ps[:])
                    ab = abuf[:, na, :]
                    l1 = l1_pool.tile([128, 2, Q], BF16, tag="l1")
                    nc.vector.tensor_tensor(
                        l1[:], ab[0:128, 0:2 * Q].rearrange(
                            "p (l w) -> p l w", l=2),
                        ab[0:128, 2 * Q:4 * Q].rearrange(
                            "p (l w) -> p l w", l=2), ALU.max)
                    if trees[na] == "d":
                        nc.vector.tensor_tensor(wm[:, p, :], l1[:, 0, :],
                                                l1[:, 1, :], ALU.max)
                    else:
                        tmp = l1_pool.tile([128, Q], BF16, tag="tmpp")
                        pool_pair_max(wm[:, p, :], l1[:, 0, :], l1[:, 1, :],
                                      tmp[:])
                    na += 1
                else:
                    nc.vector.tensor_reduce(
                        wm[:, p, :],
                        ps[:].rearrange("p (t w) -> p w t", t=4),
                        mybir.AxisListType.X, ALU.max)
            eng = (nc.sync, nc.scalar)[t % 2]
            eng.dma_start(wm_out[t], wm[:].rearrange("p a b -> p (a b)"))

    nc.compile()
    return nc


def _build_prog2():
    nc = bacc.Bacc("TRN2", target_bir_lowering=False, debug=False,
                   num_devices=NCORES)
    # host pre-transposes to partition-major layouts
    geom_in = nc.dram_tensor("geomN", [128, NT, KNN, 18], F32,
                             kind="ExternalInput").ap()
    qgeom_in = nc.dram_tensor("qgeom", [128, NT, 18], F32,
                              kind="ExternalInput").ap()
    vp_in = nc.dram_tensor("vp", [128, NT, KNN], F32, kind="ExternalInput").ap()
    hw_out = nc.dram_tensor("hw", [128, NT * KNN], F32,
                            kind="ExternalOutput").ap()

    with tile.TileContext(nc) as tc, ExitStack() as ctx:
        pool = ctx.enter_context(tc.tile_pool(name="p", bufs=1))

        TS = NT * KNN
        # small inputs first so the ACT qgr replicate starts immediately;
        # geom as two large half-DMAs on separate HWDGE queues
        nc.sync.dma_start(qg := pool.tile([128, NT, 18], F32, name="qg"),
                          qgeom_in[:])
        nc.scalar.dma_start(vp := pool.tile([128, TS], F32, name="vp"),
                            vp_in[:].rearrange("p t s -> p (t s)"))
        geom = pool.tile([128, TS, 18], F32)
        H = NT // 2
        nc.sync.dma_start(
            geom[:, :H * KNN, :],
            geom_in[:, :H].rearrange("p t s c -> p (t s) c"))
        nc.scalar.dma_start(
            geom[:, H * KNN:, :],
            geom_in[:, H:].rearrange("p t s c -> p (t s) c"))

        # replicate query geometry per neighbor slot (on ACT)
        qgr = pool.tile([128, TS, 18], F32)
        nc.scalar.copy(
            qgr[:].rearrange("p (t s) c -> p t s c", t=NT),
            qg[:].unsqueeze(2).broadcast_to([128, NT, KNN, 18]))

        hwsum = pool.tile([128, TS], F32)

        def emit_a(x0, x1):
            """Stage A: geometry products for (tile, slot) range [x0, x1)."""
            nx = x1 - x0
            SH = [128, nx, 3, 3]
            xsl = slice(x0, x1)

            def uc(c):   # query edge dir comp c (varies e1)
                return qgr[:, xsl, 9 + c:18:3].unsqueeze(3).broadcast_to(SH)

            def sc(c):   # query edge start comp c
                return qgr[:, xsl, c:9:3].unsqueeze(3).broadcast_to(SH)

            def vc(c):   # neighbor edge dir comp c (varies e2)
                return geom[:, xsl, 9 + c:18:3].unsqueeze(2).broadcast_to(SH)

            def tcp(c):  # neighbor edge start comp c
                return geom[:, xsl, c:9:3].unsqueeze(2).broadcast_to(SH)

            pfx = f"e{x0}"
            m = [pool.tile(SH, F32, name=f"{pfx}_m{i}") for i in range(6)]
            dif = [pool.tile(SH, F32, name=f"{pfx}_d{i}") for i in range(3)]
            cr = [pool.tile(SH, F32, name=f"{pfx}_cr{i}") for i in range(3)]
            DV = nc.vector.tensor_tensor
            GP = nc.gpsimd.tensor_tensor
            # broadcast-AP ops: DVE only (GPSIMD rejects broadcast APs)
            for i in range(3):  # cr_i = u_{i+1} * v_{i+2} - u_{i+2} * v_{i+1}
                a, b = (i + 1) % 3, (i + 2) % 3
                DV(m[2 * i][:], uc(a), vc(b), ALU.mult)
                DV(m[2 * i + 1][:], uc(b), vc(a), ALU.mult)
            for c in range(3):
                DV(dif[c][:], tcp(c), sc(c), ALU.subtract)
            # unit-stride chain, split DVE / GPSIMD
            DV(cr[0][:], m[0][:], m[1][:], ALU.subtract)
            GP(cr[1][:], m[2][:], m[3][:], ALU.subtract)
            GP(cr[2][:], m[4][:], m[5][:], ALU.subtract)

            num = pool.tile(SH, F32, name=f"{pfx}_num")
            t0 = pool.tile(SH, F32, name=f"{pfx}_t0")
            t1 = pool.tile(SH, F32, name=f"{pfx}_t1")
            DV(num[:], dif[0][:], cr[0][:], ALU.mult)
            GP(t0[:], dif[1][:], cr[1][:], ALU.mult)
            GP(t1[:], dif[2][:], cr[2][:], ALU.mult)
            DV(num[:], num[:], t0[:], ALU.add)
            GP(num[:], num[:], t1[:], ALU.add)


            # den2 = cr0^2 + cr1^2 + cr2^2: squares on ACT
            s0 = pool.tile(SH, F32, name=f"{pfx}_s0")
            s1 = pool.tile(SH, F32, name=f"{pfx}_s1")
            s2 = pool.tile(SH, F32, name=f"{pfx}_s2")
            nc.scalar.activation(s0[:], cr[0][:], AFT.Square)
            nc.scalar.activation(s1[:], cr[1][:], AFT.Square)
            nc.scalar.activation(s2[:], cr[2][:], AFT.Square)
            GP(s0[:], s0[:], s1[:], ALU.add)
            GP(s0[:], s0[:], s2[:], ALU.add)
            # (num/eps)^2 on ACT (scale folded into the square): the hit test
            # num^2 < eps^2*den2 becomes (num/eps)^2 < den2 directly.
            num2 = pool.tile(SH, F32, name=f"{pfx}_n2")
            nc.scalar.activation(num2[:], num[:], AFT.Square,
                                 scale=float(1.0 / EPS))
            return num2, s0

        def emit_b(x0, x1, num2, s0):
            """Stage B: hit test + weighted per-slot reduction."""
            nx = x1 - x0
            SH = [128, nx, 3, 3]
            xsl = slice(x0, x1)
            pfx = f"e{x0}"
            DV = nc.vector.tensor_tensor
            hit = pool.tile(SH, F32, name=f"{pfx}_hit")
            DV(hit[:], num2[:], s0[:], ALU.is_lt)    # GPSIMD lacks compares
            hred = pool.tile([128, nx], F32, name=f"{pfx}_hr")
            nc.vector.tensor_reduce(
                hred[:], hit[:].rearrange("p s a b -> p s (a b)"),
                mybir.AxisListType.X, ALU.add)
            DV(hwsum[:, xsl], hred[:], vp[:, xsl], ALU.mult)

        NQ = 4   # quarters, software-pipelined: stage B of quarter q is
        # emitted after stage A of quarter q+1 so engine queues never block
        pend = None
        for q in range(NQ):
            x0, x1 = q * TS // NQ, (q + 1) * TS // NQ
            cur = (x0, x1) + emit_a(x0, x1)
            if pend is not None:
                emit_b(*pend)
            pend = cur
        emit_b(*pend)

        nc.sync.dma_start(hw_out[:], hwsum[:])

    nc.compile()
    return nc


_PROGS = {}


def _get_progs():
    if "p1" not in _PROGS:
        _PROGS["p1"] = _build_prog1()
        _PROGS["p2"] = _build_prog2()
    return _PROGS["p1"], _PROGS["p2"]


def _host_prep(vertices, faces, probabilities):
    V = np.ascontiguousarray(vertices, dtype=np.float32)
    Fc = np.ascontiguousarray(faces).astype(np.int64)
    P = np.ascontiguousarray(probabilities, dtype=np.float32)
    F = Fc.shape[0]

    pos = V[Fc]                                             # [F,3,3]
    bary = (pos[:, 0] + pos[:, 1] + pos[:, 2]) / np.float32(3.0)
    sq = (bary * bary).sum(-1, dtype=np.float32)

    bf = ml_dtypes.bfloat16
    bh = bary.astype(bf).astype(np.float32)
    bl = (bary - bh).astype(bf).astype(np.float32)
    sqh = sq.astype(bf).astype(np.float32)
    sql = (sq - sqh).astype(bf).astype(np.float32)

    rhs = np.zeros((KMM, FP), np.float32)
    rhs[0:3, :F] = (2.0 * bh).T
    rhs[3:6, :F] = (2.0 * bl).T
    rhs[6:9, :F] = (2.0 * bh).T
    rhs[9:12, :F] = (2.0 * bl).T
    rhs[12, :] = -1.0
    rhs[13, :] = -1.0
    rhs[14, :F] = -sqh
    rhs[15, :F] = -sql
    rhs[14, F:] = -1.0e30
    # band b at partitions [32b, 32b+16) holds candidates [b*GW, (b+1)*GW)
    rhs_bf = rhs.astype(bf)
    rhs_b = np.zeros((128, GW), bf)
    for b in range(NGRP):
        rhs_b[32 * b:32 * b + KMM] = rhs_bf[:, b * GW:(b + 1) * GW]

    lhsT = np.zeros((KMM, FP), np.float32)
    lhsT[0:3, :F] = bh.T
    lhsT[3:6, :F] = bh.T
    lhsT[6:9, :F] = bl.T
    lhsT[9:12, :F] = bl.T
    lhsT[12, :F] = sqh
    lhsT[13, :F] = sql
    lhsT[14, :] = 1.0
    lhsT[15, :] = 1.0
    lhsT_bf = lhsT.astype(bf)
    lhsT_b = np.zeros((128, FP), bf)
    for b in range(NGRP):
        lhsT_b[32 * b:32 * b + KMM] = lhsT_bf

    starts = pos[:, [0, 0, 1], :].reshape(F, 9)
    dirs = (pos[:, [1, 2, 2], :] - pos[:, [0, 0, 1], :]).reshape(F, 9)
    geo = np.zeros((FP, 18), np.float32)
    geo[:F, 0:9] = starts
    geo[:F, 9:18] = dirs

    probs_pad = np.zeros(FP, np.float32)
    probs_pad[:F] = P

    bary_pad = np.zeros((FP, 3), np.float32)
    bary_pad[:F] = bary
    sq_pad = np.zeros(FP, np.float32)
    sq_pad[:F] = sq

    in1 = []
    for c in range(NCORES):
        lo, hi = c * NR, (c + 1) * NR
        in1.append({
            "lhsT": np.ascontiguousarray(lhsT_b[:, lo:hi]),
            "rhs": rhs_b,
        })
    aux = dict(F=F, geo=geo, probs_pad=probs_pad,
               bary=bary, sq=sq, bh=bh, bl=bl, sqh=sqh, sql=sql,
               bary_pad=bary_pad, sq_pad=sq_pad)
    return in1, aux


def _exact_rows_negd2(rows, aux):
    """Replicate the device -d2 rows in f32 (bf16-split products, f32 sums)."""
    bh, bl, sqh, sql = aux["bh"], aux["bl"], aux["sqh"], aux["sql"]
    F = aux["F"]
    rows = np.asarray(rows)
    live = rows < F                     # pad query rows have all-zero terms
    rc = np.where(live, rows, 0)
    S = len(rows)
    acc = np.zeros((S, FP), np.float32)
    for qp, cp in ((bh, bh), (bl, bh), (bh, bl), (bl, bl)):
        acc[:, :F] += (2 * qp[rc] * live[:, None]) @ cp.T
    acc[:, :F] -= ((sqh[rc] + sql[rc]) * live)[:, None]
    acc[:, :F] -= (sqh + sql)[None, :F]
    acc[:, F:] = -1.0e30
    return acc


def _host_merge(res1, aux):
    """Window-max merge: resolve the top-MWIN windows per row exactly;
    value-margin fallback to a full exact recompute. Returns nbr [FP, 16]."""
    F = aux["F"]
    wm = np.empty((FP, NWIN), np.float32)
    for c in range(NCORES):
        wm[c * NR:(c + 1) * NR] = np.asarray(
            res1.results[c]["wm"]).astype(np.float32).reshape(NR, NWIN)

    part = np.argpartition(-wm, MWIN, axis=1)
    topw = part[:, :MWIN]                                   # [FP, MWIN]
    w25 = np.take_along_axis(
        wm, part[:, MWIN:MWIN + 1], axis=1)[:, 0]           # (MWIN+1)-th value
    # rank within the partition isn't sorted; w25 must be max of the rest
    rest = np.take_along_axis(wm, part[:, MWIN:], axis=1)
    w25 = rest.max(axis=1)

    # window W (0..2559) of piece p=W//256 holds candidates
    # 1024*p + (W%256) + 256*t, t=0..3 (t-major grouping)
    k = topw // 256
    w = topw % 256
    cand = (1024 * k + w)[:, :, None] + \
        (256 * np.arange(4, dtype=np.int64))[None, None, :]
    cand = cand.reshape(FP, MWIN * 4)                       # [FP, 128]

    bary_pad, sq_pad = aux["bary_pad"], aux["sq_pad"]
    dot = np.einsum("rd,rmd->rm", bary_pad, bary_pad[cand], optimize=True)
    v = 2.0 * dot - sq_pad[:, None] - sq_pad[cand]
    v = v.astype(np.float32)
    v[cand >= F] = -1.0e30

    prt = np.argpartition(-v, KNN, axis=1)[:, :KNN]
    pv = np.take_along_axis(v, prt, axis=1)
    pc = np.take_along_axis(cand, prt, axis=1)
    order = np.lexsort((pc, -pv), axis=1)
    nbr = np.take_along_axis(pc, order, axis=1)             # [FP, 16]
    nv = np.take_along_axis(pv, order, axis=1)
    v16 = nv[:, KNN - 1]

    # fallback: the (MWIN+1)-th window max (plus bf16 + recompute margin)
    # could hide an unresolved true top-16 member.
    margin = np.abs(w25) * (2.0 ** -7) + 1e-5
    suspect = np.nonzero((w25 + margin >= v16) & (np.arange(FP) < F))[0]
    if suspect.size:
        negd2 = _exact_rows_negd2(suspect, aux)
        sp = np.argpartition(-negd2, KNN, axis=1)[:, :KNN]
        spv = np.take_along_axis(negd2, sp, axis=1)
        o = np.lexsort((sp, -spv), axis=1)
        nbr[suspect] = np.take_along_axis(sp, o, axis=1)
    return nbr


def _run(vertices, faces, probabilities, trace=False, **kw):
    p1, p2 = _get_progs()
    in1, aux = _host_prep(vertices, faces, probabilities)
    res1 = run_bass_kernel_spmd(p1, in1, list(range(NCORES)), trace=trace, **kw)
    nbr = _host_merge(res1, aux)                            # [FP, 16]
    F = aux["F"]

    geo = aux["geo"]
    geomN = geo[nbr]                                        # [FP, 16, 18]
    vp = (nbr != np.arange(FP)[:, None]).astype(np.float32) \
        * aux["probs_pad"][:, None]                         # [FP, 16]

    in2 = []
    for c in range(NCORES):
        lo, hi = c * NR, (c + 1) * NR
        in2.append({
            "geomN": np.ascontiguousarray(
                geomN[lo:hi].reshape(NT, 128, KNN, 18).transpose(1, 0, 2, 3)),
            "qgeom": np.ascontiguousarray(
                geo[lo:hi].reshape(NT, 128, 18).transpose(1, 0, 2)),
            "vp": np.ascontiguousarray(
                vp[lo:hi].reshape(NT, 128, KNN).transpose(1, 0, 2)),
        })
    res2 = run_bass_kernel_spmd(p2, in2, list(range(NCORES)), trace=trace, **kw)

    total = np.float64(0.0)
    for c in range(NCORES):
        total += np.asarray(res2.results[c]["hw"], dtype=np.float64).sum()
    loss = np.float32(total / F)
    return loss, res1, res2, nbr


def run_device(vertices, faces, probabilities, trace=False, **kw):
    loss, res1, res2, _ = _run(vertices, faces, probabilities, trace=trace, **kw)
    return loss, (res1, res2)


def kernel(vertices, faces, probabilities):
    loss, *_ = _run(vertices, faces, probabilities)
    return np.array(loss, dtype=np.float32)


# revision 50
# speedup vs baseline: 2.2909x; 1.0329x over previous
"""EdgeCrossingsLoss Trainium2 kernel (8-core SPMD, data-parallel over query faces).

Two device launches (no usable on-device gather in this runtime; the host does
the small index-merge + geometry gather between the launches):

prog1 (per core, 1280 query rows = 10 tiles of 128):
  PE:  -d2[q, c] for all 10240 candidates via a K=16 bf16 hi/lo-split matmul
       (bf16 products are exact, accumulated in f32 PSUM). rhs sits in four
       16-partition bands at base partitions 0/32/64/96. PSUM pieces are
       [128, 1024] f32 (2 banks), a 4-deep ring.
  Reduction: instead of two full f32 DVE scans (max8 + max_index, the old
       bottleneck), window-4 maxima of -d2 are computed in bf16 and shipped
       whole. TRN2 limits: GPSIMD cannot touch PSUM (and lacks max/compare
       ops in this runtime); a TensorTensor reads at most one PSUM operand.
       So 4 pieces/tile drain via DVE tensor_reduce (single strided PSUM
       input, reduce AND window in one op), 6 drain via ACT copy to bf16
       SBUF followed by a 2-level contiguous-half pair-max tree on DVE (bf16
       2x mode; one level-2 per tile on GPSIMD via max(x,y)=y+relu(x-y)).
       The [128, 2560] window-max tile is DMA'd out; no on-device top-k.

host: picks the top-32 windows per row from the 2560 bf16 window maxima
      (window W holds candidates 1024*(W//256) + W%256 + 256*t, t=0..3),
      resolves the 32*4 member candidates exactly (f32), takes the exact
      top-16 with the jax tie-break. Rows where the 33rd-best window max
      could hide a true top-16 member (value margin covering bf16 rounding)
      are recomputed exactly (vectorized, ~a few % of rows). Gathers the 16
      neighbor faces' edge geometry; folds probabilities and the
      self-neighbor mask into per-(row, slot) weights.

prog2 (per core): all 1280x16 3x3 line-line crossing tests, engine-split
      across four software-pipelined slot-quarters: DVE runs the broadcast-AP
      ops, the compare and the reduction + part of the unit-stride chain,
      GPSIMD runs adds/subs/mults of the chain, ACT squares cr^2 and
      (num/eps)^2 (eps folded into the square's scale). hit =
      (num/eps)^2 < |cross|^2 (den=0 / NaN cases fall out correctly),
      weight-masked per slot and summed per (tile, slot) on device.

Host sums the 8 per-core partials and divides by num_faces.
"""
import os
import numpy as np
import ml_dtypes
from contextlib import ExitStack

import concourse.bass as bass
import concourse.tile as tile
import concourse.bacc as bacc
from concourse import mybir
from concourse.bass_utils import run_bass_kernel_spmd

F32 = mybir.dt.float32
BF16 = mybir.dt.bfloat16
U16 = mybir.dt.uint16

NCORES = 8
KNN = 16
EPS = 1e-5
FP = 10240            # padded candidate count
NR = FP // NCORES     # 1280 rows per core
NT = NR // 128        # 10 tiles of 128 rows
KMM = 16              # matmul contraction rows (bf16 hi/lo split)
NGRP = 4              # rhs partition bands (at partitions 0/32/64/96)
GW = FP // NGRP       # 2560 candidates per band
PIECE = 1024          # PSUM piece width (f32, exactly 2 banks -> 4-deep ring)
NPIECE = FP // PIECE  # 10 pieces per tile
MMCH = 512            # matmul N per instruction (one PSUM bank)
NWIN = FP // 4        # 2560 window-4 maxima per row
MWIN = 32             # host resolves top-32 windows per row

ALU = mybir.AluOpType
AFT = mybir.ActivationFunctionType


def _build_prog1():
    nc = bacc.Bacc("TRN2", target_bir_lowering=False, debug=False,
                   num_devices=NCORES)
    # band b occupies partitions [32b, 32b+16); lhsT replicated into each band
    lhsT_in = nc.dram_tensor("lhsT", [128, NR], BF16, kind="ExternalInput").ap()
    rhs_in = nc.dram_tensor("rhs", [128, GW], BF16, kind="ExternalInput").ap()
    wm_out = nc.dram_tensor("wm", [NT, 128, NWIN], BF16,
                            kind="ExternalOutput").ap()

    with tile.TileContext(nc) as tc, ExitStack() as ctx:
        const_pool = ctx.enter_context(tc.tile_pool(name="const", bufs=1))
        psum_pool = ctx.enter_context(tc.tile_pool(name="psum", bufs=4,
                                                   space="PSUM"))
        ab_pool = ctx.enter_context(tc.tile_pool(name="ab", bufs=3))
        l1_pool = ctx.enter_context(tc.tile_pool(name="l1", bufs=6))
        wm_pool = ctx.enter_context(tc.tile_pool(name="wmp", bufs=3))

        # tile 0's lhsT slice and the first rhs chunk land first so the first
        # matmul issues ~1us earlier (the HWDGE serializes DMA descriptors)
        lhsT_sb = const_pool.tile([128, NR], BF16)
        nc.sync.dma_start(lhsT_sb[:, 0:128], lhsT_in[:, 0:128])
        rhs_sb = const_pool.tile([128, GW], BF16)
        nc.scalar.dma_start(rhs_sb[:, 0:GW // 4], rhs_in[:, 0:GW // 4])
        nc.sync.dma_start(lhsT_sb[:, 128:], lhsT_in[:, 128:])
        for j in range(1, 4):
            eng = (nc.scalar, nc.sync)[j % 2]
            eng.dma_start(rhs_sb[:, j * (GW // 4):(j + 1) * (GW // 4)],
                          rhs_in[:, j * (GW // 4):(j + 1) * (GW // 4)])

        # PSUM-read rules on TRN2: GPSIMD may not touch PSUM at all, and a
        # TensorTensor may read at most ONE operand from PSUM. GPSIMD also
        # lacks max/min/compare ops in this runtime (add/sub/mult and
        # tensor_scalar max only). So: DVE tensor_reduce (single PSUM input,
        # strided [128, 256, 4] view) drains AND window-4-maxes 4 pieces per
        # tile in one op each; ACT copies the other 6 pieces to bf16 SBUF
        # where the 2-level contiguous-half pair-max tree runs on DVE (bf16
        # 2x) or on GPSIMD via the 3-op identity max(x,y) = y + relu(x-y).
        # Windows are t-major: window w of piece p holds candidates
        # 1024*p + (w%256) + 256*t, t = 0..3.
        KINDS = "DADAADADAA"                  # 6 ACT pieces, 4 DVE pieces
        # tree engine per ACT piece: d = both levels DVE; s = level 1 on DVE,
        # level 2 on GPSIMD via the exact 3-op identity max(x,y)=y+relu(x-y)
        TREES = ["dsdsdd", "dsdsdd"]

        def pool_pair_max(dst, x, y, tmp):
            nc.gpsimd.tensor_tensor(tmp, x, y, ALU.subtract)
            nc.gpsimd.tensor_scalar(tmp, tmp, 0.0, None, ALU.max)
            nc.gpsimd.tensor_tensor(dst, y, tmp, ALU.add)

        for t in range(NT):
            trees = TREES[t % 2]
            abuf = ab_pool.tile([128, 6, PIECE], BF16, tag="ab")
            wm = wm_pool.tile([128, NPIECE, PIECE // 4], BF16, tag="wm")
            na = 0
            for p in range(NPIECE):
                ps = psum_pool.tile([128, PIECE], F32, tag="ps")
                for c0 in range(0, PIECE, MMCH):
                    gcol = p * PIECE + c0      # global candidate column
                    g = gcol // GW             # band
                    off = gcol - g * GW
                    nc.tensor.matmul(
                        ps[:, c0:c0 + MMCH],
                        lhsT=lhsT_sb[32 * g:32 * g + KMM,
                                     t * 128:(t + 1) * 128],
                        rhs=rhs_sb[32 * g:32 * g + KMM, off:off + MMCH],
                        start=True, stop=True,
                        tile_position=(32 * g, 0),
                    )
                Q = PIECE // 4
                if KINDS[p] == "A":
                    nc.scalar.copy(abuf[:, na, :], ps[:])
                    ab = abuf[:, na, :]
                    l1 = l1_pool.tile([128, 2, Q], BF16, tag="l1")
                    nc.vector.tensor_tensor(
                        l1[:], ab[0:128, 0:2 * Q].rearrange(
                            "p (l w) -> p l w", l=2),
                        ab[0:128, 2 * Q:4 * Q].rearrange(
                            "p (l w) -> p l w", l=2), ALU.max)
                    if trees[na] == "d":
                        nc.vector.tensor_tensor(wm[:, p, :], l1[:, 0, :],
                                                l1[:, 1, :], ALU.max)
                    else:
                        tmp = l1_pool.tile([128, Q], BF16, tag="tmpp")
                        pool_pair_max(wm[:, p, :], l1[:, 0, :], l1[:, 1, :],
                                      tmp[:])
                    na += 1
                else:
                    nc.vector.tensor_reduce(
                        wm[:, p, :],
                        ps[:].rearrange("p (t w) -> p w t", t=4),
                        mybir.AxisListType.X, ALU.max)
            eng = (nc.sync, nc.scalar)[t % 2]
            eng.dma_start(wm_out[t], wm[:].rearrange("p a b -> p (a b)"))

    nc.compile()
    return nc


def _build_prog2():
    nc = bacc.Bacc("TRN2", target_bir_lowering=False, debug=False,
                   num_devices=NCORES)
    # host pre-transposes to partition-major layouts
    geom_in = nc.dram_tensor("geomN", [128, NT, KNN, 18], F32,
                             kind="ExternalInput").ap()
    qgeom_in = nc.dram_tensor("qgeom", [128, NT, 18], F32,
                              kind="ExternalInput").ap()
    vp_in = nc.dram_tensor("vp", [128, NT, KNN], F32, kind="ExternalInput").ap()
    hw_out = nc.dram_tensor("hw", [128, NT * KNN], F32,
                            kind="ExternalOutput").ap()

    with tile.TileContext(nc) as tc, ExitStack() as ctx:
        pool = ctx.enter_context(tc.tile_pool(name="p", bufs=1))

        TS = NT * KNN
        NQ = 4   # 40-slot chunks
        # qg first (it gates every chunk's broadcast ops), then geom chunk by
        # chunk on alternating queues so chunk q's compute starts as soon as
        # its own slice lands; vp last (only stage B reads it). The HWDGE
        # serializes DMA descriptors (~630ns each), so chunk 0 is ready ~2.5us
        # in instead of waiting for the whole geom tensor.
        nc.sync.dma_start(qg := pool.tile([128, NT, 18], F32, name="qg"),
                          qgeom_in[:])
        geom = pool.tile([128, TS, 18], F32)
        geom_flat = geom_in[:].rearrange("p t s c -> p (t s) c")
        QW = TS // NQ
        for q in range(NQ):
            eng = (nc.scalar, nc.sync)[q % 2]
            eng.dma_start(geom[:, q * QW:(q + 1) * QW, :],
                          geom_flat[:, q * QW:(q + 1) * QW, :])
        nc.sync.dma_start(vp := pool.tile([128, TS], F32, name="vp"),
                          vp_in[:].rearrange("p t s -> p (t s)"))

        # replicate query geometry per neighbor slot (on ACT), in two
        # tile-aligned halves so the first chunk is gated by half the work
        qgr = pool.tile([128, TS, 18], F32)
        H = NT // 2
        nc.scalar.copy(
            qgr[:, :H * KNN, :].rearrange("p (t s) c -> p t s c", t=H),
            qg[:, :H].unsqueeze(2).broadcast_to([128, H, KNN, 18]))
        nc.scalar.copy(
            qgr[:, H * KNN:, :].rearrange("p (t s) c -> p t s c", t=H),
            qg[:, H:].unsqueeze(2).broadcast_to([128, H, KNN, 18]))

        hwsum = pool.tile([128, TS], F32)

        def mk_views(x0, x1):
            nx = x1 - x0
            SH = [128, nx, 3, 3]
            xsl = slice(x0, x1)

            def uc(c):   # query edge dir comp c (varies e1)
                return qgr[:, xsl, 9 + c:18:3].unsqueeze(3).broadcast_to(SH)

            def sc(c):   # query edge start comp c
                return qgr[:, xsl, c:9:3].unsqueeze(3).broadcast_to(SH)

            def vc(c):   # neighbor edge dir comp c (varies e2)
                return geom[:, xsl, 9 + c:18:3].unsqueeze(2).broadcast_to(SH)

            def tcp(c):  # neighbor edge start comp c
                return geom[:, xsl, c:9:3].unsqueeze(2).broadcast_to(SH)

            return SH, xsl, uc, sc, vc, tcp

        DV = nc.vector.tensor_tensor
        GP = nc.gpsimd.tensor_tensor

        def emit_a1(x0, x1):
            """Stage A1: broadcast products (DVE-only), ordered so the
            GPSIMD consumers (cr1/cr2 then t0/t1) unblock earliest."""
            SH, xsl, uc, sc, vc, tcp = mk_views(x0, x1)
            pfx = f"e{x0}"
            m = [pool.tile(SH, F32, name=f"{pfx}_m{i}") for i in range(6)]
            dif = [pool.tile(SH, F32, name=f"{pfx}_d{i}") for i in range(3)]
            for i in (1, 2, 0):  # cr_i = u_{i+1}*v_{i+2} - u_{i+2}*v_{i+1}
                a, b = (i + 1) % 3, (i + 2) % 3
                DV(m[2 * i][:], uc(a), vc(b), ALU.mult)
                DV(m[2 * i + 1][:], uc(b), vc(a), ALU.mult)
                if i != 0:
                    GP_cr(pfx, SH, m, i)
            for c in (1, 2, 0):
                DV(dif[c][:], tcp(c), sc(c), ALU.subtract)
            return m, dif

        _cr = {}

        def GP_cr(pfx, SH, m, i):
            cr = pool.tile(SH, F32, name=f"{pfx}_cr{i}")
            GP(cr[:], m[2 * i][:], m[2 * i + 1][:], ALU.subtract)
            _cr[(pfx, i)] = cr

        def emit_a2(x0, x1, m, dif):
            """Stage A2: the unit-stride num/den chain (DVE+GPSIMD+ACT)."""
            SH, xsl, *_ = mk_views(x0, x1)
            pfx = f"e{x0}"
            cr0 = pool.tile(SH, F32, name=f"{pfx}_cr0")
            DV(cr0[:], m[0][:], m[1][:], ALU.subtract)
            cr1, cr2 = _cr[(pfx, 1)], _cr[(pfx, 2)]

            num = pool.tile(SH, F32, name=f"{pfx}_num")
            t0 = pool.tile(SH, F32, name=f"{pfx}_t0")
            t1 = pool.tile(SH, F32, name=f"{pfx}_t1")
            GP(t0[:], dif[1][:], cr1[:], ALU.mult)
            GP(t1[:], dif[2][:], cr2[:], ALU.mult)
            DV(num[:], dif[0][:], cr0[:], ALU.mult)
            DV(num[:], num[:], t0[:], ALU.add)
            GP(num[:], num[:], t1[:], ALU.add)

            # den2 = cr0^2 + cr1^2 + cr2^2: squares on ACT
            s0 = pool.tile(SH, F32, name=f"{pfx}_s0")
            s1 = pool.tile(SH, F32, name=f"{pfx}_s1")
            s2 = pool.tile(SH, F32, name=f"{pfx}_s2")
            nc.scalar.activation(s1[:], cr1[:], AFT.Square)
            nc.scalar.activation(s2[:], cr2[:], AFT.Square)
            nc.scalar.activation(s0[:], cr0[:], AFT.Square)
            GP(s0[:], s0[:], s1[:], ALU.add)
            GP(s0[:], s0[:], s2[:], ALU.add)
            # (num/eps)^2 on ACT (scale folded into the square): the hit test
            # num^2 < eps^2*den2 becomes (num/eps)^2 < den2 directly.
            num2 = pool.tile(SH, F32, name=f"{pfx}_n2")
            nc.scalar.activation(num2[:], num[:], AFT.Square,
                                 scale=float(1.0 / EPS))
            return num2, s0

        def emit_b(x0, x1, num2, s0):
            """Stage B: hit test + weighted per-slot reduction."""
            nx = x1 - x0
            SH = [128, nx, 3, 3]
            xsl = slice(x0, x1)
            pfx = f"e{x0}"
            hit = pool.tile(SH, F32, name=f"{pfx}_hit")
            DV(hit[:], num2[:], s0[:], ALU.is_lt)    # GPSIMD lacks compares
            hred = pool.tile([128, nx], F32, name=f"{pfx}_hr")
            nc.vector.tensor_reduce(
                hred[:], hit[:].rearrange("p s a b -> p s (a b)"),
                mybir.AxisListType.X, ALU.add)
            DV(hwsum[:, xsl], hred[:], vp[:, xsl], ALU.mult)

        # 3-stage software pipeline: A1(q) | A2(q-1) | B(q-2)
        bounds = [(q * TS // NQ, (q + 1) * TS // NQ) for q in range(NQ)]
        st_a = {}
        st_b = {}
        for q in range(NQ + 2):
            if q < NQ:
                st_a[q] = emit_a1(*bounds[q])
            if 1 <= q <= NQ:
                st_b[q - 1] = emit_a2(*bounds[q - 1], *st_a.pop(q - 1))
            if q >= 2:
                emit_b(*bounds[q - 2], *st_b.pop(q - 2))

        nc.sync.dma_start(hw_out[:], hwsum[:])

    nc.compile()
    return nc


_PROGS = {}


def _get_progs():
    if "p1" not in _PROGS:
        _PROGS["p1"] = _build_prog1()
        _PROGS["p2"] = _build_prog2()
    return _PROGS["p1"], _PROGS["p2"]


def _host_prep(vertices, faces, probabilities):
    V = np.ascontiguousarray(vertices, dtype=np.float32)
    Fc = np.ascontiguousarray(faces).astype(np.int64)
    P = np.ascontiguousarray(probabilities, dtype=np.float32)
    F = Fc.shape[0]

    pos = V[Fc]                                             # [F,3,3]
    bary = (pos[:, 0] + pos[:, 1] + pos[:, 2]) / np.float32(3.0)
    sq = (bary * bary).sum(-1, dtype=np.float32)

    bf = ml_dtypes.bfloat16
    bh = bary.astype(bf).astype(np.float32)
    bl = (bary - bh).astype(bf).astype(np.float32)
    sqh = sq.astype(bf).astype(np.float32)
    sql = (sq - sqh).astype(bf).astype(np.float32)

    rhs = np.zeros((KMM, FP), np.float32)
    rhs[0:3, :F] = (2.0 * bh).T
    rhs[3:6, :F] = (2.0 * bl).T
    rhs[6:9, :F] = (2.0 * bh).T
    rhs[9:12, :F] = (2.0 * bl).T
    rhs[12, :] = -1.0
    rhs[13, :] = -1.0
    rhs[14, :F] = -sqh
    rhs[15, :F] = -sql
    rhs[14, F:] = -1.0e30
    # band b at partitions [32b, 32b+16) holds candidates [b*GW, (b+1)*GW)
    rhs_bf = rhs.astype(bf)
    rhs_b = np.zeros((128, GW), bf)
    for b in range(NGRP):
        rhs_b[32 * b:32 * b + KMM] = rhs_bf[:, b * GW:(b + 1) * GW]

    lhsT = np.zeros((KMM, FP), np.float32)
    lhsT[0:3, :F] = bh.T
    lhsT[3:6, :F] = bh.T
    lhsT[6:9, :F] = bl.T
    lhsT[9:12, :F] = bl.T
    lhsT[12, :F] = sqh
    lhsT[13, :F] = sql
    lhsT[14, :] = 1.0
    lhsT[15, :] = 1.0
    lhsT_bf = lhsT.astype(bf)
    lhsT_b = np.zeros((128, FP), bf)
    for b in range(NGRP):
        lhsT_b[32 * b:32 * b + KMM] = lhsT_bf

    starts = pos[:, [0, 0, 1], :].reshape(F, 9)
    dirs = (pos[:, [1, 2, 2], :] - pos[:, [0, 0, 1], :]).reshape(F, 9)
    geo = np.zeros((FP, 18), np.float32)
    geo[:F, 0:9] = starts
    geo[:F, 9:18] = dirs

    probs_pad = np.zeros(FP, np.float32)
    probs_pad[:F] = P

    bary_pad = np.zeros((FP, 3), np.float32)
    bary_pad[:F] = bary
    sq_pad = np.zeros(FP, np.float32)
    sq_pad[:F] = sq

    in1 = []
    for c in range(NCORES):
        lo, hi = c * NR, (c + 1) * NR
        in1.append({
            "lhsT": np.ascontiguousarray(lhsT_b[:, lo:hi]),
            "rhs": rhs_b,
        })
    aux = dict(F=F, geo=geo, probs_pad=probs_pad,
               bary=bary, sq=sq, bh=bh, bl=bl, sqh=sqh, sql=sql,
               bary_pad=bary_pad, sq_pad=sq_pad)
    return in1, aux


def _exact_rows_negd2(rows, aux):
    """Replicate the device -d2 rows in f32 (bf16-split products, f32 sums)."""
    bh, bl, sqh, sql = aux["bh"], aux["bl"], aux["sqh"], aux["sql"]
    F = aux["F"]
    rows = np.asarray(rows)
    live = rows < F                     # pad query rows have all-zero terms
    rc = np.where(live, rows, 0)
    S = len(rows)
    acc = np.zeros((S, FP), np.float32)
    for qp, cp in ((bh, bh), (bl, bh), (bh, bl), (bl, bl)):
        acc[:, :F] += (2 * qp[rc] * live[:, None]) @ cp.T
    acc[:, :F] -= ((sqh[rc] + sql[rc]) * live)[:, None]
    acc[:, :F] -= (sqh + sql)[None, :F]
    acc[:, F:] = -1.0e30
    return acc


def _host_merge(res1, aux):
    """Window-max merge: resolve the top-MWIN windows per row exactly;
    value-margin fallback to a full exact recompute. Returns nbr [FP, 16]."""
    F = aux["F"]
    wm = np.empty((FP, NWIN), np.float32)
    for c in range(NCORES):
        wm[c * NR:(c + 1) * NR] = np.asarray(
            res1.results[c]["wm"]).astype(np.float32).reshape(NR, NWIN)

    part = np.argpartition(-wm, MWIN, axis=1)
    topw = part[:, :MWIN]                                   # [FP, MWIN]
    w25 = np.take_along_axis(
        wm, part[:, MWIN:MWIN + 1], axis=1)[:, 0]           # (MWIN+1)-th value
    # rank within the partition isn't sorted; w25 must be max of the rest
    rest = np.take_along_axis(wm, part[:, MWIN:], axis=1)
    w25 = rest.max(axis=1)

    # window W (0..2559) of piece p=W//256 holds candidates
    # 1024*p + (W%256) + 256*t, t=0..3 (t-major grouping)
    k = topw // 256
    w = topw % 256
    cand = (1024 * k + w)[:, :, None] + \
        (256 * np.arange(4, dtype=np.int64))[None, None, :]
    cand = cand.reshape(FP, MWIN * 4)                       # [FP, 128]

    bary_pad, sq_pad = aux["bary_pad"], aux["sq_pad"]
    dot = np.einsum("rd,rmd->rm", bary_pad, bary_pad[cand], optimize=True)
    v = 2.0 * dot - sq_pad[:, None] - sq_pad[cand]
    v = v.astype(np.float32)
    v[cand >= F] = -1.0e30

    prt = np.argpartition(-v, KNN, axis=1)[:, :KNN]
    pv = np.take_along_axis(v, prt, axis=1)
    pc = np.take_along_axis(cand, prt, axis=1)
    order = np.lexsort((pc, -pv), axis=1)
    nbr = np.take_along_axis(pc, order, axis=1)             # [FP, 16]
    nv = np.take_along_axis(pv, order, axis=1)
    v16 = nv[:, KNN - 1]

    # fallback: the (MWIN+1)-th window max (plus bf16 + recompute margin)
    # could hide an unresolved true top-16 member.
    margin = np.abs(w25) * (2.0 ** -7) + 1e-5
    suspect = np.nonzero((w25 + margin >= v16) & (np.arange(FP) < F))[0]
    if suspect.size:
        negd2 = _exact_rows_negd2(suspect, aux)
        sp = np.argpartition(-negd2, KNN, axis=1)[:, :KNN]
        spv = np.take_along_axis(negd2, sp, axis=1)
        o = np.lexsort((sp, -spv), axis=1)
        nbr[suspect] = np.take_along_axis(sp, o, axis=1)
    return nbr


def _run(vertices, faces, probabilities, trace=False, **kw):
    p1, p2 = _get_progs()
    in1, aux = _host_prep(vertices, faces, probabilities)
    res1 = run_bass_kernel_spmd(p1, in1, list(range(NCORES)), trace=trace, **kw)
    nbr = _host_merge(res1, aux)                            # [FP, 16]
    F = aux["F"]

    geo = aux["geo"]
    geomN = geo[nbr]                                        # [FP, 16, 18]
    vp = (nbr != np.arange(FP)[:, None]).astype(np.float32) \
        * aux["probs_pad"][:, None]                         # [FP, 16]

    in2 = []
    for c in range(NCORES):
        lo, hi = c * NR, (c + 1) * NR
        in2.append({
            "geomN": np.ascontiguousarray(
                geomN[lo:hi].reshape(NT, 128, KNN, 18).transpose(1, 0, 2, 3)),
            "qgeom": np.ascontiguousarray(
                geo[lo:hi].reshape(NT, 128, 18).transpose(1, 0, 2)),
            "vp": np.ascontiguousarray(
                vp[lo:hi].reshape(NT, 128, KNN).transpose(1, 0, 2)),
        })
    res2 = run_bass_kernel_spmd(p2, in2, list(range(NCORES)), trace=trace, **kw)

    total = np.float64(0.0)
    for c in range(NCORES):
        total += np.asarray(res2.results[c]["hw"], dtype=np.float64).sum()
    loss = np.float32(total / F)
    return loss, res1, res2, nbr


def run_device(vertices, faces, probabilities, trace=False, **kw):
    loss, res1, res2, _ = _run(vertices, faces, probabilities, trace=trace, **kw)
    return loss, (res1, res2)


def kernel(vertices, faces, probabilities):
    loss, *_ = _run(vertices, faces, probabilities)
    return np.array(loss, dtype=np.float32)
